# revision 5
# baseline (speedup 1.0000x reference)
"""AVSL similarity kernel for Trainium2 (8 NeuronCores, data-parallel over B1).

Math (per (b1,b2) pair, d-vector chain over 3 layers):
  n_l = (normalize(emb1_l[b1]) - normalize(emb2_l[b2]))**2        [D]
  hat_0 = n_0
  hat_l = (1-P_l) * (hat_{l-1} @ W_l) + P_l * n_l,  l=1,2
  P_l   = sigmoid(alpha_l * cert1_l[b1] * cert2_l[b2] + beta_l)
  W_l   = col-top3-masked, col-normalized link_{l-1}
  out[b1,b2] = sum_d hat_2

Device decomposition (everything in [d(=128 partitions), b2(=512 free)] layout):
  mm1 = W1^T n0                      (PE)
  u1  = P1 * (n1 - mm1)              (ACT sigmoid/square + DVE)
  mm2 = W12^T n0 + W2^T u1           (PE; W12 = W1@W2 precomputed on-chip)
  u2  = P2 * (n2 - mm2)
  out_row = w12s^T n0 + w2s^T u1 + 1^T u2   (PE M=1 matmuls; w*s = row sums)

Sharding: emb1/cert1 rows split 64/core; emb2/cert2/links/alpha/beta replicated.
"""
import os
import sys

sys.path.insert(0, "/opt/trn_rl_repo")

import numpy as np

import concourse.bass as bass
import concourse.tile as tile
from concourse import bacc, mybir
from concourse.bass_utils import run_bass_kernel_spmd

N_CORES = 8
B1, B2, D = 512, 512, 128
RPC = B1 // N_CORES  # rows of ovr_sim per core
F32 = mybir.dt.float32
AF = mybir.ActivationFunctionType
OP = mybir.AluOpType
AX = mybir.AxisListType

_cache = {}


def _transpose_norm_512(nc, pre, pps, const, ident, dram_ap, tag, normalize):
    """Load a [512,128] DRAM tensor, optionally l2-normalize rows, return
    SBUF [128(d), 512(b2)] transposed tile."""
    tp = pps.tile([128, 512], F32, tag="tp512")
    for blk in range(4):
        t = pre.tile([128, 128], F32, tag="ld")
        nc.sync.dma_start(t[:], dram_ap[blk * 128 : (blk + 1) * 128, :])
        if normalize:
            sq = pre.tile([128, 128], F32, tag="sq")
            nc.vector.tensor_mul(sq[:], t[:], t[:])
            ss = pre.tile([128, 1], F32, tag="ss")
            nc.vector.reduce_sum(ss[:], sq[:], axis=AX.X)
            nrm = pre.tile([128, 1], F32, tag="nrm")
            nc.scalar.sqrt(nrm[:], ss[:])
            nrm2 = pre.tile([128, 1], F32, tag="nrm2")
            nc.vector.tensor_scalar_max(nrm2[:], nrm[:], 1e-12)
            rn = pre.tile([128, 1], F32, tag="rn")
            nc.vector.reciprocal(rn[:], nrm2[:])
            tn = pre.tile([128, 128], F32, tag="tn")
            nc.vector.tensor_scalar_mul(tn[:], t[:], rn[:])
            t = tn
        nc.tensor.transpose(tp[:, blk * 128 : (blk + 1) * 128], t[:], ident[:])
    out = const.tile([128, 512], F32, tag=tag)
    nc.scalar.copy(out[:], tp[:])
    return out


def _prep_link(nc, pre, pps, const, ident, dram_ap, i):
    """Top-3-per-column mask + column-normalize of link [d,e].
    Returns (WnT [e,d] SBUF, Wn [d,e] SBUF)."""
    lt = pre.tile([128, 128], F32, tag="wld")
    nc.sync.dma_start(lt[:], dram_ap[:, :])
    tpw = pps.tile([128, 128], F32, tag="tpw")
    nc.tensor.transpose(tpw[:], lt[:], ident[:])
    wt = pre.tile([128, 128], F32, tag="wt")
    nc.scalar.copy(wt[:], tpw[:])  # [e, d]

    x = wt
    m = None
    for k in range(3):
        m = pre.tile([128, 1], F32, tag=f"wm{k}")
        nc.vector.reduce_max(m[:], x[:], axis=AX.X)
        if k < 2:
            msk = pre.tile([128, 128], F32, tag=f"wmask{k}")
            # ((x >= m) * -2) + x : push current max below everything
            nc.vector.tensor_scalar(msk[:], x[:], m[:], -2.0, op0=OP.is_ge, op1=OP.mult)
            x2 = pre.tile([128, 128], F32, tag=f"wx{k}")
            nc.vector.tensor_add(x2[:], x[:], msk[:])
            x = x2
    # m now holds the 3rd-largest original value per row; keep entries >= m
    wm = pre.tile([128, 128], F32, tag="wkeep")
    nc.vector.scalar_tensor_tensor(wm[:], wt[:], m[:], wt[:], op0=OP.is_ge, op1=OP.mult)
    cs = pre.tile([128, 1], F32, tag="wcs")
    nc.vector.reduce_sum(cs[:], wm[:], axis=AX.X)
    cse = pre.tile([128, 1], F32, tag="wcse")
    nc.vector.tensor_scalar_add(cse[:], cs[:], 1e-8)
    rc = pre.tile([128, 1], F32, tag="wrc")
    nc.vector.reciprocal(rc[:], cse[:])
    wnT = const.tile([128, 128], F32, tag=f"wnT{i}")
    nc.vector.tensor_scalar_mul(wnT[:], wm[:], rc[:])  # [e, d]
    tpw2 = pps.tile([128, 128], F32, tag="tpw")
    nc.tensor.transpose(tpw2[:], wnT[:], ident[:])
    wn = const.tile([128, 128], F32, tag=f"Wn{i}")
    nc.scalar.copy(wn[:], tpw2[:])  # [d, e]
    return wnT, wn


def _build():
    nc = bacc.Bacc("TRN2", target_bir_lowering=False, debug=False)
    de1 = [nc.dram_tensor(f"emb1_{l}", [RPC, D], F32, kind="ExternalInput") for l in range(3)]
    dc1 = [nc.dram_tensor(f"cert1_{l}", [RPC, D], F32, kind="ExternalInput") for l in (1, 2)]
    de2 = [nc.dram_tensor(f"emb2_{l}", [B2, D], F32, kind="ExternalInput") for l in range(3)]
    dc2 = [nc.dram_tensor(f"cert2_{l}", [B2, D], F32, kind="ExternalInput") for l in (1, 2)]
    dal = [nc.dram_tensor(f"alpha_{l}", [D, 1], F32, kind="ExternalInput") for l in (1, 2)]
    dbe = [nc.dram_tensor(f"beta_{l}", [D, 1], F32, kind="ExternalInput") for l in (1, 2)]
    dlk = [nc.dram_tensor(f"link_{l}", [D, D], F32, kind="ExternalInput") for l in range(2)]
    did = nc.dram_tensor("ident", [D, D], F32, kind="ExternalInput")
    dout = nc.dram_tensor("ovr", [RPC, B2], F32, kind="ExternalOutput")

    with tile.TileContext(nc) as tc:
        with tc.tile_pool(name="const", bufs=1) as const:
            ident = const.tile([128, 128], F32, tag="ident")
            nc.sync.dma_start(ident[:], did.ap())
            acol = []
            bcol = []
            for i in range(2):
                a = const.tile([128, 1], F32, tag=f"acol{i}")
                nc.sync.dma_start(a[:], dal[i].ap())
                acol.append(a)
                b = const.tile([128, 1], F32, tag=f"bcol{i}")
                nc.sync.dma_start(b[:], dbe[i].ap())
                bcol.append(b)

            e2T = [None] * 3
            c2T = [None] * 2
            ne1T = [None] * 3
            scT = [None] * 2
            with tc.tile_pool(name="pre", bufs=4) as pre, tc.tile_pool(
                name="prepsum", bufs=2, space="PSUM"
            ) as pps:
                for l in range(3):
                    e2T[l] = _transpose_norm_512(
                        nc, pre, pps, const, ident, de2[l].ap(), f"e2T{l}", True
                    )
                for i in range(2):
                    c2T[i] = _transpose_norm_512(
                        nc, pre, pps, const, ident, dc2[i].ap(), f"c2T{i}", False
                    )
                # emb1 shard: normalize rows, negate, transpose -> [d, r]
                for l in range(3):
                    t = pre.tile([64, 128], F32, tag="e1ld")
                    nc.sync.dma_start(t[:], de1[l].ap())
                    sq = pre.tile([64, 128], F32, tag="e1sq")
                    nc.vector.tensor_mul(sq[:], t[:], t[:])
                    ss = pre.tile([64, 1], F32, tag="e1ss")
                    nc.vector.reduce_sum(ss[:], sq[:], axis=AX.X)
                    nrm = pre.tile([64, 1], F32, tag="e1nrm")
                    nc.scalar.sqrt(nrm[:], ss[:])
                    nrm2 = pre.tile([64, 1], F32, tag="e1nrm2")
                    nc.vector.tensor_scalar_max(nrm2[:], nrm[:], 1e-12)
                    rn = pre.tile([64, 1], F32, tag="e1rn")
                    nc.vector.reciprocal(rn[:], nrm2[:])
                    rneg = pre.tile([64, 1], F32, tag="e1rneg")
                    nc.scalar.mul(rneg[:], rn[:], -1.0)
                    tn = pre.tile([64, 128], F32, tag="e1tn")
                    nc.vector.tensor_scalar_mul(tn[:], t[:], rneg[:])
                    tp = pps.tile([128, 64], F32, tag="tp64")
                    nc.tensor.transpose(tp[:], tn[:], ident[:64, :64])
                    ne1T[l] = const.tile([128, 64], F32, tag=f"ne1T{l}", name=f"ne1T{l}")
                    nc.scalar.copy(ne1T[l][:], tp[:])
                # cert1 shard: transpose, scale by alpha -> [d, r]
                for i in range(2):
                    t = pre.tile([64, 128], F32, tag="c1ld")
                    nc.sync.dma_start(t[:], dc1[i].ap())
                    tp = pps.tile([128, 64], F32, tag="tp64")
                    nc.tensor.transpose(tp[:], t[:], ident[:64, :64])
                    c1T = pre.tile([128, 64], F32, tag="c1T")
                    nc.scalar.copy(c1T[:], tp[:])
                    scT[i] = const.tile([128, 64], F32, tag=f"scT{i}", name=f"scT{i}")
                    nc.vector.tensor_scalar_mul(scT[i][:], c1T[:], acol[i][:])
                # links
                wnT0, Wn0 = _prep_link(nc, pre, pps, const, ident, dlk[0].ap(), 0)
                _, Wn1 = _prep_link(nc, pre, pps, const, ident, dlk[1].ap(), 1)
                # W12 = W1 @ W2  ([d,g]); row sums
                tpw = pps.tile([128, 128], F32, tag="tpw")
                nc.tensor.matmul(tpw[:], lhsT=wnT0[:], rhs=Wn1[:], start=True, stop=True)
                W12 = const.tile([128, 128], F32, tag="W12")
                nc.scalar.copy(W12[:], tpw[:])
                w12s = const.tile([128, 1], F32, tag="w12s")
                nc.vector.reduce_sum(w12s[:], W12[:], axis=AX.X)
                w2s = const.tile([128, 1], F32, tag="w2s")
                nc.vector.reduce_sum(w2s[:], Wn1[:], axis=AX.X)
                ones = const.tile([128, 1], F32, tag="ones")
                nc.vector.memset(ones[:], 1.0)

            with tc.tile_pool(name="row", bufs=3) as rowp, tc.tile_pool(
                name="psA", bufs=2, space="PSUM"
            ) as psA, tc.tile_pool(name="psB", bufs=2, space="PSUM") as psB, tc.tile_pool(
                name="psC", bufs=2, space="PSUM"
            ) as psC:
                for r in range(RPC):
                    n0 = rowp.tile([128, 512], F32, tag="n0")
                    nc.scalar.activation(
                        n0[:], e2T[0][:], AF.Square, bias=ne1T[0][:, r : r + 1]
                    )
                    A = psA.tile([128, 512], F32, tag="A")
                    nc.tensor.matmul(A[:], lhsT=Wn0[:], rhs=n0[:], start=True, stop=True)
                    n1 = rowp.tile([128, 512], F32, tag="n1")
                    nc.scalar.activation(
                        n1[:], e2T[1][:], AF.Square, bias=ne1T[1][:, r : r + 1]
                    )
                    P1 = rowp.tile([128, 512], F32, tag="P1")
                    nc.scalar.activation(
                        P1[:],
                        c2T[0][:],
                        AF.Sigmoid,
                        bias=bcol[0][:],
                        scale=scT[0][:, r : r + 1],
                    )
                    s1 = rowp.tile([128, 512], F32, tag="s1")
                    nc.vector.tensor_sub(s1[:], n1[:], A[:])
                    u1 = rowp.tile([128, 512], F32, tag="u1")
                    nc.vector.tensor_mul(u1[:], s1[:], P1[:])
                    Bp = psB.tile([128, 512], F32, tag="B")
                    nc.tensor.matmul(Bp[:], lhsT=W12[:], rhs=n0[:], start=True, stop=False)
                    nc.tensor.matmul(Bp[:], lhsT=Wn1[:], rhs=u1[:], start=False, stop=True)
                    n2 = rowp.tile([128, 512], F32, tag="n2")
                    nc.scalar.activation(
                        n2[:], e2T[2][:], AF.Square, bias=ne1T[2][:, r : r + 1]
                    )
                    P2 = rowp.tile([128, 512], F32, tag="P2")
                    nc.scalar.activation(
                        P2[:],
                        c2T[1][:],
                        AF.Sigmoid,
                        bias=bcol[1][:],
                        scale=scT[1][:, r : r + 1],
                    )
                    s2 = rowp.tile([128, 512], F32, tag="s2")
                    nc.vector.tensor_sub(s2[:], n2[:], Bp[:])
                    u2 = rowp.tile([128, 512], F32, tag="u2")
                    nc.vector.tensor_mul(u2[:], s2[:], P2[:])
                    C = psC.tile([1, 512], F32, tag="C")
                    nc.tensor.matmul(C[:], lhsT=w12s[:], rhs=n0[:], start=True, stop=False)
                    nc.tensor.matmul(C[:], lhsT=w2s[:], rhs=u1[:], start=False, stop=False)
                    nc.tensor.matmul(C[:], lhsT=ones[:], rhs=u2[:], start=False, stop=True)
                    stag = rowp.tile([1, 512], F32, tag="stag")
                    nc.vector.tensor_copy(stag[:], C[:])
                    nc.sync.dma_start(dout.ap()[r : r + 1, :], stag[:])
    nc.compile()
    return nc


def _get_nc():
    if "nc" not in _cache:
        _cache["nc"] = _build()
    return _cache["nc"]


def kernel(**inputs):
    nc = _get_nc()
    ident = np.eye(D, dtype=np.float32)
    in_maps = []
    for c in range(N_CORES):
        sl = slice(c * RPC, (c + 1) * RPC)
        m = {"ident": ident}
        for l in range(3):
            m[f"emb1_{l}"] = np.ascontiguousarray(inputs[f"emb1_{l}"][sl])
            m[f"emb2_{l}"] = np.asarray(inputs[f"emb2_{l}"])
        for l in (1, 2):
            m[f"cert1_{l}"] = np.ascontiguousarray(inputs[f"cert1_{l}"][sl])
            m[f"cert2_{l}"] = np.asarray(inputs[f"cert2_{l}"])
            m[f"alpha_{l}"] = np.asarray(inputs[f"alpha_{l}"]).reshape(D, 1)
            m[f"beta_{l}"] = np.asarray(inputs[f"beta_{l}"]).reshape(D, 1)
        for l in range(2):
            m[f"link_{l}"] = np.asarray(inputs[f"link_{l}"])
        in_maps.append(m)
    trace = bool(int(os.environ.get("AVSL_TRACE", "0")))
    res = run_bass_kernel_spmd(nc, in_maps, core_ids=list(range(N_CORES)), trace=trace)
    _cache["last_result"] = res
    return np.concatenate([res.results[c]["ovr"] for c in range(N_CORES)], axis=0)


# revision 6
# speedup vs baseline: 1.9609x; 1.9609x over previous
"""AVSL similarity kernel for Trainium2 (8 NeuronCores, data-parallel over B1).

Math (per (b1,b2) pair, d-vector chain over 3 layers):
  n_l = (normalize(emb1_l[b1]) - normalize(emb2_l[b2]))**2        [D]
  hat_0 = n_0
  hat_l = (1-P_l) * (hat_{l-1} @ W_l) + P_l * n_l,  l=1,2
  P_l   = sigmoid(alpha_l * cert1_l[b1] * cert2_l[b2] + beta_l)
  W_l   = col-top3-masked, col-normalized link_{l-1}
  out[b1,b2] = sum_d hat_2

Device decomposition, [d(=128 partitions), b2(=512 free)] layout, Q_l = 1-P_l
(computed directly as sigmoid of the negated argument):
  A  = n1 - W1^T n0                       (PE only: negated weights + I*n1)
  v1 = Q1 * A          => hat1 = n1 - v1  (DVE; Q via ACT)
  B  = n2 - W2^T n1 + W2^T v1  (= n2 - mm2)   (PE)
  v2 = Q2 * B          => hat2 = n2 - v2  (DVE)
  out_row = 1^T n2 - 1^T v2               (PE M=1 matmuls)
All matmul operands bf16 (single HW pass); PSUM stays fp32.

Sharding: emb1/cert1 rows split 64/core; emb2/cert2/links/alpha/beta replicated.
"""
import os
import sys

sys.path.insert(0, "/opt/trn_rl_repo")

import numpy as np

import concourse.bass as bass
import concourse.tile as tile
from concourse import bacc, mybir
from concourse.bass_utils import run_bass_kernel_spmd

N_CORES = 8
B1, B2, D = 512, 512, 128
RPC = B1 // N_CORES  # rows of ovr_sim per core
F32 = mybir.dt.float32
BF16 = mybir.dt.bfloat16
AF = mybir.ActivationFunctionType
OP = mybir.AluOpType
AX = mybir.AxisListType

_cache = {}


def _transpose_norm_512(nc, pre, pps, const, ident, dram_ap, tag, normalize):
    """Load a [512,128] DRAM tensor, optionally l2-normalize rows, return
    SBUF [128(d), 512(b2)] transposed tile (fp32)."""
    tp = pps.tile([128, 512], F32, tag="tp512")
    for blk in range(4):
        t = pre.tile([128, 128], F32, tag="ld")
        nc.sync.dma_start(t[:], dram_ap[blk * 128 : (blk + 1) * 128, :])
        if normalize:
            sq = pre.tile([128, 128], F32, tag="sq")
            nc.vector.tensor_mul(sq[:], t[:], t[:])
            ss = pre.tile([128, 1], F32, tag="ss")
            nc.vector.reduce_sum(ss[:], sq[:], axis=AX.X)
            nrm = pre.tile([128, 1], F32, tag="nrm")
            nc.scalar.sqrt(nrm[:], ss[:])
            nrm2 = pre.tile([128, 1], F32, tag="nrm2")
            nc.vector.tensor_scalar_max(nrm2[:], nrm[:], 1e-12)
            rn = pre.tile([128, 1], F32, tag="rn")
            nc.vector.reciprocal(rn[:], nrm2[:])
            tn = pre.tile([128, 128], F32, tag="tn")
            nc.vector.tensor_scalar_mul(tn[:], t[:], rn[:])
            t = tn
        nc.tensor.transpose(tp[:, blk * 128 : (blk + 1) * 128], t[:], ident[:])
    out = const.tile([128, 512], F32, tag=tag)
    nc.scalar.copy(out[:], tp[:])
    return out


def _prep_link(nc, pre, pps, const, ident, dram_ap, i, want_pos):
    """Top-3-per-column mask + column-normalize of link [d,e].
    Returns (negW bf16 [d,e], W bf16 [d,e] or None)."""
    lt = pre.tile([128, 128], F32, tag="wld")
    nc.sync.dma_start(lt[:], dram_ap[:, :])
    tpw = pps.tile([128, 128], F32, tag="tpw")
    nc.tensor.transpose(tpw[:], lt[:], ident[:])
    wt = pre.tile([128, 128], F32, tag="wt")
    nc.scalar.copy(wt[:], tpw[:])  # [e, d]

    x = wt
    m = None
    for k in range(3):
        m = pre.tile([128, 1], F32, tag=f"wm{k}")
        nc.vector.reduce_max(m[:], x[:], axis=AX.X)
        if k < 2:
            msk = pre.tile([128, 128], F32, tag=f"wmask{k}")
            # ((x >= m) * -2) + x : push current max below everything
            nc.vector.tensor_scalar(msk[:], x[:], m[:], -2.0, op0=OP.is_ge, op1=OP.mult)
            x2 = pre.tile([128, 128], F32, tag=f"wx{k}")
            nc.vector.tensor_add(x2[:], x[:], msk[:])
            x = x2
    # m = 3rd-largest original value per row; keep entries >= m
    wm = pre.tile([128, 128], F32, tag="wkeep")
    nc.vector.scalar_tensor_tensor(wm[:], wt[:], m[:], wt[:], op0=OP.is_ge, op1=OP.mult)
    cs = pre.tile([128, 1], F32, tag="wcs")
    nc.vector.reduce_sum(cs[:], wm[:], axis=AX.X)
    cse = pre.tile([128, 1], F32, tag="wcse")
    nc.vector.tensor_scalar_add(cse[:], cs[:], 1e-8)
    rc = pre.tile([128, 1], F32, tag="wrc")
    nc.vector.reciprocal(rc[:], cse[:])
    nrc = pre.tile([128, 1], F32, tag="wnrc")
    nc.scalar.mul(nrc[:], rc[:], -1.0)
    # negated normalized W^T, then transpose back to [d,e]
    wnT = pre.tile([128, 128], F32, tag=f"wnT{i}", name=f"wnT{i}")
    nc.vector.tensor_scalar_mul(wnT[:], wm[:], nrc[:])  # [e, d] (negated)
    tpw2 = pps.tile([128, 128], F32, tag="tpw")
    nc.tensor.transpose(tpw2[:], wnT[:], ident[:])
    negw = const.tile([128, 128], BF16, tag=f"negW{i}", name=f"negW{i}")
    nc.scalar.copy(negw[:], tpw2[:])  # [d, e] bf16, negated
    posw = None
    if want_pos:
        posw = const.tile([128, 128], BF16, tag=f"posW{i}", name=f"posW{i}")
        nc.scalar.mul(posw[:], tpw2[:], -1.0)  # [d, e] bf16, positive
    return negw, posw


def _build():
    nc = bacc.Bacc("TRN2", target_bir_lowering=False, debug=False)
    de1 = [nc.dram_tensor(f"emb1_{l}", [RPC, D], F32, kind="ExternalInput") for l in range(3)]
    dc1 = [nc.dram_tensor(f"cert1_{l}", [RPC, D], F32, kind="ExternalInput") for l in (1, 2)]
    de2 = [nc.dram_tensor(f"emb2_{l}", [B2, D], F32, kind="ExternalInput") for l in range(3)]
    dc2 = [nc.dram_tensor(f"cert2_{l}", [B2, D], F32, kind="ExternalInput") for l in (1, 2)]
    dal = [nc.dram_tensor(f"alpha_{l}", [D, 1], F32, kind="ExternalInput") for l in (1, 2)]
    dbe = [nc.dram_tensor(f"beta_{l}", [D, 1], F32, kind="ExternalInput") for l in (1, 2)]
    dlk = [nc.dram_tensor(f"link_{l}", [D, D], F32, kind="ExternalInput") for l in range(2)]
    did = nc.dram_tensor("ident", [D, D], F32, kind="ExternalInput")
    dout = nc.dram_tensor("ovr", [RPC, B2], F32, kind="ExternalOutput")

    with tile.TileContext(nc) as tc:
        with tc.tile_pool(name="const", bufs=1) as const:
            ident = const.tile([128, 128], F32, tag="ident")
            nc.sync.dma_start(ident[:], did.ap())
            identb = const.tile([128, 128], BF16, tag="identb")
            nc.vector.tensor_copy(identb[:], ident[:])
            onesb = const.tile([128, 1], BF16, tag="onesb")
            nc.vector.memset(onesb[:], 1.0)
            negonesb = const.tile([128, 1], BF16, tag="negonesb")
            nc.vector.memset(negonesb[:], -1.0)
            nacol = []  # -alpha_l column
            nbcol = []  # -beta_l column
            for i in range(2):
                a = const.tile([128, 1], F32, tag=f"acol{i}", name=f"acol{i}")
                nc.sync.dma_start(a[:], dal[i].ap())
                na = const.tile([128, 1], F32, tag=f"nacol{i}", name=f"nacol{i}")
                nc.scalar.mul(na[:], a[:], -1.0)
                nacol.append(na)
                b = const.tile([128, 1], F32, tag=f"bcol{i}", name=f"bcol{i}")
                nc.sync.dma_start(b[:], dbe[i].ap())
                nb = const.tile([128, 1], F32, tag=f"nbcol{i}", name=f"nbcol{i}")
                nc.scalar.mul(nb[:], b[:], -1.0)
                nbcol.append(nb)

            e2T = [None] * 3
            c2T = [None] * 2
            ne1T = [None] * 3
            nscT = [None] * 2
            with tc.tile_pool(name="pre", bufs=4) as pre, tc.tile_pool(
                name="prepsum", bufs=2, space="PSUM"
            ) as pps:
                for l in range(3):
                    e2T[l] = _transpose_norm_512(
                        nc, pre, pps, const, ident, de2[l].ap(), f"e2T{l}", True
                    )
                for i in range(2):
                    c2T[i] = _transpose_norm_512(
                        nc, pre, pps, const, ident, dc2[i].ap(), f"c2T{i}", False
                    )
                # emb1 shard: normalize rows, negate, transpose -> [d, r]
                for l in range(3):
                    t = pre.tile([64, 128], F32, tag="e1ld")
                    nc.sync.dma_start(t[:], de1[l].ap())
                    sq = pre.tile([64, 128], F32, tag="e1sq")
                    nc.vector.tensor_mul(sq[:], t[:], t[:])
                    ss = pre.tile([64, 1], F32, tag="e1ss")
                    nc.vector.reduce_sum(ss[:], sq[:], axis=AX.X)
                    nrm = pre.tile([64, 1], F32, tag="e1nrm")
                    nc.scalar.sqrt(nrm[:], ss[:])
                    nrm2 = pre.tile([64, 1], F32, tag="e1nrm2")
                    nc.vector.tensor_scalar_max(nrm2[:], nrm[:], 1e-12)
                    rn = pre.tile([64, 1], F32, tag="e1rn")
                    nc.vector.reciprocal(rn[:], nrm2[:])
                    rneg = pre.tile([64, 1], F32, tag="e1rneg")
                    nc.scalar.mul(rneg[:], rn[:], -1.0)
                    tn = pre.tile([64, 128], F32, tag="e1tn")
                    nc.vector.tensor_scalar_mul(tn[:], t[:], rneg[:])
                    tp = pps.tile([128, 64], F32, tag="tp64")
                    nc.tensor.transpose(tp[:], tn[:], ident[:64, :64])
                    ne1T[l] = const.tile([128, 64], F32, tag=f"ne1T{l}", name=f"ne1T{l}")
                    nc.scalar.copy(ne1T[l][:], tp[:])
                # cert1 shard: transpose, scale by -alpha -> [d, r]
                for i in range(2):
                    t = pre.tile([64, 128], F32, tag="c1ld")
                    nc.sync.dma_start(t[:], dc1[i].ap())
                    tp = pps.tile([128, 64], F32, tag="tp64")
                    nc.tensor.transpose(tp[:], t[:], ident[:64, :64])
                    c1T = pre.tile([128, 64], F32, tag="c1T")
                    nc.scalar.copy(c1T[:], tp[:])
                    nscT[i] = const.tile([128, 64], F32, tag=f"nscT{i}", name=f"nscT{i}")
                    nc.vector.tensor_scalar_mul(nscT[i][:], c1T[:], nacol[i][:])
                # links: W1 only negated; W2 both signs
                negW1, _ = _prep_link(nc, pre, pps, const, ident, dlk[0].ap(), 0, False)
                negW2, posW2 = _prep_link(nc, pre, pps, const, ident, dlk[1].ap(), 1, True)

            with tc.tile_pool(name="row", bufs=3) as rowp, tc.tile_pool(
                name="psA", bufs=2, space="PSUM"
            ) as psA, tc.tile_pool(name="psB", bufs=2, space="PSUM") as psB, tc.tile_pool(
                name="psC", bufs=2, space="PSUM"
            ) as psC:
                for r in range(RPC):
                    n0 = rowp.tile([128, 512], BF16, tag="n0")
                    nc.scalar.activation(
                        n0[:], e2T[0][:], AF.Square, bias=ne1T[0][:, r : r + 1]
                    )
                    n1 = rowp.tile([128, 512], BF16, tag="n1")
                    nc.scalar.activation(
                        n1[:], e2T[1][:], AF.Square, bias=ne1T[1][:, r : r + 1]
                    )
                    A = psA.tile([128, 512], F32, tag="A")
                    nc.tensor.matmul(A[:], lhsT=negW1[:], rhs=n0[:], start=True, stop=False)
                    nc.tensor.matmul(A[:], lhsT=identb[:], rhs=n1[:], start=False, stop=True)
                    Q1 = rowp.tile([128, 512], F32, tag="Q1")
                    nc.scalar.activation(
                        Q1[:],
                        c2T[0][:],
                        AF.Sigmoid,
                        bias=nbcol[0][:],
                        scale=nscT[0][:, r : r + 1],
                    )
                    v1 = rowp.tile([128, 512], BF16, tag="v1")
                    nc.vector.tensor_mul(v1[:], Q1[:], A[:])
                    n2 = rowp.tile([128, 512], BF16, tag="n2")
                    nc.scalar.activation(
                        n2[:], e2T[2][:], AF.Square, bias=ne1T[2][:, r : r + 1]
                    )
                    Bp = psB.tile([128, 512], F32, tag="B")
                    nc.tensor.matmul(Bp[:], lhsT=negW2[:], rhs=n1[:], start=True, stop=False)
                    nc.tensor.matmul(Bp[:], lhsT=posW2[:], rhs=v1[:], start=False, stop=False)
                    nc.tensor.matmul(Bp[:], lhsT=identb[:], rhs=n2[:], start=False, stop=True)
                    Q2 = rowp.tile([128, 512], F32, tag="Q2")
                    nc.scalar.activation(
                        Q2[:],
                        c2T[1][:],
                        AF.Sigmoid,
                        bias=nbcol[1][:],
                        scale=nscT[1][:, r : r + 1],
                    )
                    v2 = rowp.tile([128, 512], BF16, tag="v2")
                    nc.vector.tensor_mul(v2[:], Q2[:], Bp[:])
                    C = psC.tile([1, 512], F32, tag="C")
                    nc.tensor.matmul(C[:], lhsT=onesb[:], rhs=n2[:], start=True, stop=False)
                    nc.tensor.matmul(C[:], lhsT=negonesb[:], rhs=v2[:], start=False, stop=True)
                    stag = rowp.tile([1, 512], F32, tag="stag")
                    nc.vector.tensor_copy(stag[:], C[:])
                    nc.sync.dma_start(dout.ap()[r : r + 1, :], stag[:])
    nc.compile()
    return nc


def _get_nc():
    if "nc" not in _cache:
        _cache["nc"] = _build()
    return _cache["nc"]


def kernel(**inputs):
    nc = _get_nc()
    ident = np.eye(D, dtype=np.float32)
    in_maps = []
    for c in range(N_CORES):
        sl = slice(c * RPC, (c + 1) * RPC)
        m = {"ident": ident}
        for l in range(3):
            m[f"emb1_{l}"] = np.ascontiguousarray(inputs[f"emb1_{l}"][sl])
            m[f"emb2_{l}"] = np.asarray(inputs[f"emb2_{l}"])
        for l in (1, 2):
            m[f"cert1_{l}"] = np.ascontiguousarray(inputs[f"cert1_{l}"][sl])
            m[f"cert2_{l}"] = np.asarray(inputs[f"cert2_{l}"])
            m[f"alpha_{l}"] = np.asarray(inputs[f"alpha_{l}"]).reshape(D, 1)
            m[f"beta_{l}"] = np.asarray(inputs[f"beta_{l}"]).reshape(D, 1)
        for l in range(2):
            m[f"link_{l}"] = np.asarray(inputs[f"link_{l}"])
        in_maps.append(m)
    trace = bool(int(os.environ.get("AVSL_TRACE", "0")))
    res = run_bass_kernel_spmd(nc, in_maps, core_ids=list(range(N_CORES)), trace=trace)
    _cache["last_result"] = res
    return np.concatenate([res.results[c]["ovr"] for c in range(N_CORES)], axis=0)
